# revision 2
# baseline (speedup 1.0000x reference)
"""CTRGC kernel: pure data-parallel over 8 NeuronCores.

Shards batch N=256 across the 8 cores (32 each); the small conv weights
(w1..w4, b1..b4), alpha, and the [V,V] adjacency A are replicated.
Each core computes its batch shard; results are gathered to full shape.

Self-contained: hardcodes shapes N,C,T,V = 256,64,64,25 / REL=8 / OUT=64.
"""

import jax
import jax.numpy as jnp
import numpy as np

N, C, T, V = 256, 64, 64, 25
REL, OUT = 8, 64
N_CORES = 8


def _forward(x, A, alpha, w1, b1, w2, b2, w3, b3, w4, b4):
    # x: [n_shard, C, T, V] on one core
    xm = x.mean(axis=2)                                            # [n, C, V]
    x1 = jnp.einsum('ncv,rc->nrv', xm, w1) + b1[None, :, None]     # [n, R, V]
    x2 = jnp.einsum('ncv,rc->nrv', xm, w2) + b2[None, :, None]     # [n, R, V]
    x3 = jnp.einsum('nctv,oc->notv', x, w3) + b3[None, :, None, None]
    # affT[n,r,v,u] = tanh(x1[u] - x2[v]): v-major so the final matmul
    # contracts x3's last dim against Mt's second-to-last with no transpose
    affT = jnp.tanh(x1[:, :, None, :] - x2[:, :, :, None])         # [n, R, V(v), V(u)]
    Mt = jnp.einsum('nrvu,or->novu', affT, w4) + b4[None, :, None, None]
    Mt = Mt * alpha + A.T[None, None]                              # Mt[n,o,v,u] = M[n,o,u,v]
    out = jnp.einsum('notv,novu->notu', x3, Mt)                    # [n, O, T, V]
    return out


_pforward = jax.pmap(
    _forward,
    in_axes=(0,) + (None,) * 10,   # shard x on batch; replicate the rest
    out_axes=0,
)


def kernel(x, A, alpha, w1, b1, w2, b2, w3, b3, w4, b4):
    x = np.asarray(x, dtype=np.float32)
    shard = N // N_CORES
    xs = x.reshape(N_CORES, shard, C, T, V)
    args = [np.asarray(a, dtype=np.float32)
            for a in (A, alpha, w1, b1, w2, b2, w3, b3, w4, b4)]
    out = _pforward(xs, *args)                # [8, 32, O, T, V]
    out = np.asarray(out).reshape(N, OUT, T, V).astype(np.float32)
    return out
